# revision 1
# baseline (speedup 1.0000x reference)
"""Trainium2 Bass kernel for nn_MergedLinearFormer.

Computes out = softmax((x@QK)@x^T / sqrt(D)) @ x @ VO for x:[B,T,D].

Sharding: 8 cores; core c handles batch b=c//2, query half h=c%2 (2048
queries each). Inside a core, everything is computed with the score matrix
TRANSPOSED (keys on PSUM partitions, queries on the free axis) so that no
on-chip transposes are needed anywhere:

  phase 1:  xQK^T[e, q]   = QK^T @ xq^T          (lhsT=QK,  rhs=x^T cols)
  S-phase:  S^T[u, q]     = x @ xQK^T             (lhsT=x^T, rhs=xQK^T)
            P^T[u, q]     = exp(S^T / sqrt(D))    (no max subtraction:
                             scores are ~N(0,1), exp can't overflow)
            den[q]       += ones^T @ P^T          (matmul with ones lhsT)
  AV-phase: av^T[d, q]    = x^T @ P^T             (lhsT=x,   rhs=P^T)
  OUT:      out[q, e]     = (av^T)^T @ VO         (lhsT=av^T, rhs=VO)
            out[q, e]    /= den[q]

All matmul operands are bf16 (PE streams 1 column/cycle regardless of
dtype, so bf16 halves DMA/SBUF at no PE cost); accumulation is fp32 in
PSUM; output fp32.
"""

import numpy as np
import ml_dtypes

import concourse.bass as bass
import concourse.mybir as mybir
import concourse.tile as tile
from concourse import bacc
from concourse.bass_utils import run_bass_kernel_spmd

P = 128
B, T, D = 4, 4096, 1024
TQ = T // 2          # queries per core
CH = 512             # query-chunk width
ET = D // P          # 8 tiles along the model dim
UT = T // P          # 32 tiles along the key dim
CHUNKS = TQ // CH    # 4
JT = CH // P         # 4 query tiles per chunk
SCALE = 1.0 / np.sqrt(D)

BF16 = mybir.dt.bfloat16
F32 = mybir.dt.float32
NPBF16 = ml_dtypes.bfloat16


def _build():
    nc = bacc.Bacc()
    xT = nc.dram_tensor("xT", [D, T], BF16, kind="ExternalInput")
    x_ = nc.dram_tensor("x", [T, D], BF16, kind="ExternalInput")
    xTq = nc.dram_tensor("xTq", [D, TQ], BF16, kind="ExternalInput")
    QK = nc.dram_tensor("QK", [D, D], BF16, kind="ExternalInput")
    VO = nc.dram_tensor("VO", [D, D], BF16, kind="ExternalInput")
    out = nc.dram_tensor("out", [TQ, D], F32, kind="ExternalOutput")

    xT_r = xT.rearrange("(eo p) u -> p eo u", p=P)      # [128, 8, 4096]
    x_r = x_.rearrange("(uo p) d -> p uo d", p=P)       # [128, 32, 1024]
    xTq_r = xTq.rearrange("(eo p) q -> p eo q", p=P)    # [128, 8, 2048]
    QK_r = QK.rearrange("(ko p) e -> p ko e", p=P)      # [128, 8, 1024]
    VO_r = VO.rearrange("(ko p) e -> p ko e", p=P)      # [128, 8, 1024]

    with tile.TileContext(nc) as tc:
        with tc.tile_pool(name="xqkt_pool", bufs=1) as xqkt_pool:
            xqkt = xqkt_pool.tile([P, ET, TQ], BF16)  # resident: xQK^T

            # ---- phase 1: xQK^T[e, q] ----
            with (
                tc.tile_pool(name="ph1", bufs=1) as ph1,
                tc.tile_pool(name="ph1ps", bufs=4, space="PSUM") as ph1ps,
            ):
                qk_sb = ph1.tile([P, ET, D], BF16)
                nc.sync.dma_start(qk_sb, QK_r)
                xtq_sb = ph1.tile([P, ET, TQ], BF16)
                nc.sync.dma_start(xtq_sb, xTq_r)
                for et in range(ET):
                    for nq in range(TQ // 512):
                        ps = ph1ps.tile([P, 512], F32, name="ph1_ps")
                        for kt in range(ET):
                            nc.tensor.matmul(
                                ps,
                                qk_sb[:, kt, et * P : (et + 1) * P],
                                xtq_sb[:, kt, nq * 512 : (nq + 1) * 512],
                                start=(kt == 0),
                                stop=(kt == ET - 1),
                            )
                        nc.scalar.copy(xqkt[:, et, nq * 512 : (nq + 1) * 512], ps)

            # ---- main pools ----
            with (
                tc.tile_pool(name="consts", bufs=1) as consts,
                tc.tile_pool(name="ptpool", bufs=UT + 2) as ptpool,
                tc.tile_pool(name="xtpan_pool", bufs=3) as xtpan_pool,
                tc.tile_pool(name="xpan_pool", bufs=2) as xpan_pool,
                tc.tile_pool(name="avpool", bufs=2) as avpool,
                tc.tile_pool(name="outpool", bufs=2) as outpool,
                tc.tile_pool(name="small", bufs=2) as small,
                tc.tile_pool(name="ps_s", bufs=2, space="PSUM") as ps_s_pool,
                tc.tile_pool(name="ps_sums", bufs=2, space="PSUM") as ps_sums_pool,
                tc.tile_pool(name="ps_av", bufs=2, space="PSUM") as ps_av_pool,
                tc.tile_pool(name="ps_o", bufs=2, space="PSUM") as ps_o_pool,
            ):
                ones_sb = consts.tile([P, 1], BF16)
                nc.vector.memset(ones_sb, 1.0)
                vo_sb = consts.tile([P, ET, D], BF16)
                nc.sync.dma_start(vo_sb, VO_r)

                for c in range(CHUNKS):
                    q0 = c * CH
                    # ---- S-phase: S^T tiles, exp, denominator accumulation ----
                    sums_ps = ps_sums_pool.tile([1, CH], F32, name="sums_ps")
                    pts = []
                    for ut in range(UT):
                        xt_pan = xtpan_pool.tile([P, ET, P], BF16, name="xt_pan")
                        nc.sync.dma_start(xt_pan, xT_r[:, :, ut * P : (ut + 1) * P])
                        s_ps = ps_s_pool.tile([P, CH], F32, name="s_ps")
                        for kt in range(ET):
                            nc.tensor.matmul(
                                s_ps,
                                xt_pan[:, kt, :],
                                xqkt[:, kt, q0 : q0 + CH],
                                start=(kt == 0),
                                stop=(kt == ET - 1),
                            )
                        pt = ptpool.tile([P, CH], BF16, name="pt")
                        nc.scalar.activation(
                            pt, s_ps, mybir.ActivationFunctionType.Exp, scale=SCALE
                        )
                        nc.tensor.matmul(
                            sums_ps,
                            ones_sb,
                            pt,
                            start=(ut == 0),
                            stop=(ut == UT - 1),
                        )
                        pts.append(pt)

                    # ---- denominators -> per-partition reciprocals ----
                    rec_sb = small.tile([1, CH], F32, name="rec_sb")
                    nc.vector.reciprocal(rec_sb, sums_ps)
                    r_sb = small.tile([P, JT], F32, name="r_sb")
                    for j in range(JT):
                        nc.sync.dma_start(
                            r_sb[:, j : j + 1], rec_sb[0:1, j * P : (j + 1) * P]
                        )

                    # ---- AV-phase: av^T[d, q] ----
                    av_sb = avpool.tile([P, ET, CH], BF16, name="av_sb")
                    for dt in range(ET):
                        x_pan = xpan_pool.tile([P, UT, P], BF16, name="x_pan")
                        nc.sync.dma_start(x_pan, x_r[:, :, dt * P : (dt + 1) * P])
                        av_ps = ps_av_pool.tile([P, CH], F32, name="av_ps")
                        for ut in range(UT):
                            nc.tensor.matmul(
                                av_ps,
                                x_pan[:, ut, :],
                                pts[ut],
                                start=(ut == 0),
                                stop=(ut == UT - 1),
                            )
                        nc.vector.tensor_copy(av_sb[:, dt, :], av_ps)

                    # ---- OUT: (av^T)^T @ VO, normalized ----
                    for j in range(JT):
                        out_sb = outpool.tile([P, D], F32, name="out_sb")
                        for eh in range(2):
                            o_ps = ps_o_pool.tile([P, 512], F32, name="o_ps")
                            for dt in range(ET):
                                nc.tensor.matmul(
                                    o_ps,
                                    av_sb[:, dt, j * P : (j + 1) * P],
                                    vo_sb[:, dt, eh * 512 : (eh + 1) * 512],
                                    start=(dt == 0),
                                    stop=(dt == ET - 1),
                                )
                            nc.vector.tensor_scalar_mul(
                                out_sb[:, eh * 512 : (eh + 1) * 512],
                                o_ps,
                                r_sb[:, j : j + 1],
                            )
                        nc.sync.dma_start(
                            out[q0 + j * P : q0 + (j + 1) * P, :], out_sb
                        )

    nc.compile()
    return nc


_NC = None


def _get_nc():
    global _NC
    if _NC is None:
        _NC = _build()
    return _NC


def kernel(x, QK, VO):
    x = np.asarray(x, dtype=np.float32)
    QK16 = np.asarray(QK, dtype=np.float32).astype(NPBF16)
    VO16 = np.asarray(VO, dtype=np.float32).astype(NPBF16)

    in_maps = []
    for c in range(8):
        b, h = divmod(c, 2)
        xb16 = x[b].astype(NPBF16)                         # [T, D]
        xTb16 = np.ascontiguousarray(xb16.T)               # [D, T]
        in_maps.append(
            {
                "x": xb16,
                "xT": xTb16,
                "xTq": np.ascontiguousarray(xTb16[:, h * TQ : (h + 1) * TQ]),
                "QK": QK16,
                "VO": VO16,
            }
        )

    res = run_bass_kernel_spmd(_get_nc(), in_maps, core_ids=list(range(8)))

    out = np.empty((B, T, D), dtype=np.float32)
    for c in range(8):
        b, h = divmod(c, 2)
        out[b, h * TQ : (h + 1) * TQ, :] = res.results[c]["out"]
    return out


# revision 5
# speedup vs baseline: 1.7258x; 1.7258x over previous
"""Trainium2 Bass kernel for nn_MergedLinearFormer.

Computes out = softmax((x@QK)@x^T / sqrt(D)) @ x @ VO for x:[B,T,D].

Sharding: 8 cores; core c handles batch b=c//2, query half h=c%2 (2048
queries each). Inside a core, everything is computed with the score matrix
TRANSPOSED (keys on PSUM partitions, queries on the free axis) so that no
on-chip transposes are needed anywhere:

  phase 1:  xQK^T[e, q]   = QK^T @ xq^T          (lhsT=QK,  rhs=x^T cols)
  S-phase:  S^T[u, q]     = x @ xQK^T             (lhsT=x^T, rhs=xQK^T)
            P^T[u, q]     = exp(S^T / sqrt(D))    (no max subtraction:
                             scores are ~N(0,1), exp can't overflow)
            den[q]       += ones^T @ P^T          (matmul with ones lhsT)
  AV-phase: av^T[d, q]    = x^T @ P^T             (lhsT=x,   rhs=P^T)
  OUT:      out[q, e]     = (av^T)^T @ VO         (lhsT=av^T, rhs=VO)
            out[q, e]    /= den[q]

All matmul operands are bf16 (PE streams 1 column/cycle regardless of
dtype, so bf16 halves DMA/SBUF at no PE cost); accumulation is fp32 in
PSUM; output fp32.
"""

import os

import numpy as np
import ml_dtypes

import concourse.bass as bass
import concourse.mybir as mybir
import concourse.tile as tile
from concourse import bacc

P = 128
B, T, D = 4, 4096, 1024
TQ = T // 2          # queries per core
CH = 512             # query-chunk width
ET = D // P          # 8 tiles along the model dim
UT = T // P          # 32 tiles along the key dim
CHUNKS = TQ // CH    # 4
JT = CH // P         # 4 query tiles per chunk
SCALE = 1.0 / np.sqrt(D)

BF16 = mybir.dt.bfloat16
F32 = mybir.dt.float32
NPBF16 = ml_dtypes.bfloat16


def _build():
    # REPEAT>1 re-runs the whole main phase (identical results) so benchmarks
    # can fit a per-iteration slope that cancels fixed dispatch overhead.
    repeat = int(os.environ.get("BASS_KERNEL_REPEAT", "1"))
    nc = bacc.Bacc()
    xT = nc.dram_tensor("xT", [D, T], BF16, kind="ExternalInput")
    x_ = nc.dram_tensor("x", [T, D], BF16, kind="ExternalInput")
    xTq = nc.dram_tensor("xTq", [D, TQ], BF16, kind="ExternalInput")
    QK = nc.dram_tensor("QK", [D, D], BF16, kind="ExternalInput")
    VO = nc.dram_tensor("VO", [D, D], BF16, kind="ExternalInput")
    out = nc.dram_tensor("out", [TQ, D], F32, kind="ExternalOutput")

    xT_r = xT.rearrange("(eo p) u -> p eo u", p=P)      # [128, 8, 4096]
    x_r = x_.rearrange("(uo p) d -> p uo d", p=P)       # [128, 32, 1024]
    xTq_r = xTq.rearrange("(eo p) q -> p eo q", p=P)    # [128, 8, 2048]
    QK_r = QK.rearrange("(ko p) e -> p ko e", p=P)      # [128, 8, 1024]
    VO_r = VO.rearrange("(ko p) e -> p ko e", p=P)      # [128, 8, 1024]

    with tile.TileContext(nc) as tc:
        with tc.tile_pool(name="xqkt_pool", bufs=1) as xqkt_pool:
            xqkt = xqkt_pool.tile([P, ET, TQ], BF16)  # resident: xQK^T

            # ---- phase 1: xQK^T[e, q] ----
            with (
                tc.tile_pool(name="ph1", bufs=1) as ph1,
                tc.tile_pool(name="ph1ps", bufs=4, space="PSUM") as ph1ps,
            ):
                qk_sb = ph1.tile([P, ET, D], BF16)
                nc.sync.dma_start(qk_sb, QK_r)
                xtq_sb = ph1.tile([P, ET, TQ], BF16)
                nc.sync.dma_start(xtq_sb, xTq_r)
                for et in range(ET):
                    for nq in range(TQ // 512):
                        ps = ph1ps.tile([P, 512], F32, name="ph1_ps")
                        for kt in range(ET):
                            nc.tensor.matmul(
                                ps,
                                qk_sb[:, kt, et * P : (et + 1) * P],
                                xtq_sb[:, kt, nq * 512 : (nq + 1) * 512],
                                start=(kt == 0),
                                stop=(kt == ET - 1),
                            )
                        nc.scalar.copy(xqkt[:, et, nq * 512 : (nq + 1) * 512], ps)

            # ---- main pools ----
            with (
                tc.tile_pool(name="consts", bufs=1) as consts,
                tc.tile_pool(name="ptpool", bufs=UT + 2) as ptpool,
                tc.tile_pool(name="xtpan_pool", bufs=3) as xtpan_pool,
                tc.tile_pool(name="xpan_pool", bufs=2) as xpan_pool,
                tc.tile_pool(name="avpool", bufs=2) as avpool,
                tc.tile_pool(name="outpool", bufs=2) as outpool,
                tc.tile_pool(name="small", bufs=2) as small,
                tc.tile_pool(name="ps_s", bufs=2, space="PSUM") as ps_s_pool,
                tc.tile_pool(name="ps_sums", bufs=2, space="PSUM") as ps_sums_pool,
                tc.tile_pool(name="ps_av", bufs=2, space="PSUM") as ps_av_pool,
                tc.tile_pool(name="ps_o", bufs=2, space="PSUM") as ps_o_pool,
            ):
                ones_sb = consts.tile([P, 1], BF16)
                nc.vector.memset(ones_sb, 1.0)
                vo_sb = consts.tile([P, ET, D], BF16)
                nc.sync.dma_start(vo_sb, VO_r)

                for c in range(CHUNKS * repeat):
                    c = c % CHUNKS
                    q0 = c * CH
                    # ---- S-phase: S^T tiles, exp, denominator accumulation ----
                    sums_ps = ps_sums_pool.tile([1, CH], F32, name="sums_ps")
                    pts = []
                    for ut in range(UT):
                        xt_pan = xtpan_pool.tile([P, ET, P], BF16, name="xt_pan")
                        nc.sync.dma_start(xt_pan, xT_r[:, :, ut * P : (ut + 1) * P])
                        s_ps = ps_s_pool.tile([P, CH], F32, name="s_ps")
                        for kt in range(ET):
                            nc.tensor.matmul(
                                s_ps,
                                xt_pan[:, kt, :],
                                xqkt[:, kt, q0 : q0 + CH],
                                start=(kt == 0),
                                stop=(kt == ET - 1),
                            )
                        pt = ptpool.tile([P, CH], BF16, name="pt")
                        nc.scalar.activation(
                            pt, s_ps, mybir.ActivationFunctionType.Exp, scale=SCALE
                        )
                        nc.tensor.matmul(
                            sums_ps,
                            ones_sb,
                            pt,
                            start=(ut == 0),
                            stop=(ut == UT - 1),
                        )
                        pts.append(pt)

                    # ---- denominators -> per-partition reciprocals ----
                    rec_sb = small.tile([1, CH], F32, name="rec_sb")
                    nc.vector.reciprocal(rec_sb, sums_ps)
                    r_sb = small.tile([P, JT], F32, name="r_sb")
                    for j in range(JT):
                        nc.sync.dma_start(
                            r_sb[:, j : j + 1], rec_sb[0:1, j * P : (j + 1) * P]
                        )

                    # ---- AV-phase: av^T[d, q] ----
                    av_sb = avpool.tile([P, ET, CH], BF16, name="av_sb")
                    for dt in range(ET):
                        x_pan = xpan_pool.tile([P, UT, P], BF16, name="x_pan")
                        nc.sync.dma_start(x_pan, x_r[:, :, dt * P : (dt + 1) * P])
                        av_ps = ps_av_pool.tile([P, CH], F32, name="av_ps")
                        for ut in range(UT):
                            nc.tensor.matmul(
                                av_ps,
                                x_pan[:, ut, :],
                                pts[ut],
                                start=(ut == 0),
                                stop=(ut == UT - 1),
                            )
                        nc.vector.tensor_copy(av_sb[:, dt, :], av_ps)

                    # ---- OUT: (av^T)^T @ VO, normalized ----
                    for j in range(JT):
                        out_sb = outpool.tile([P, D], F32, name="out_sb")
                        for eh in range(2):
                            o_ps = ps_o_pool.tile([P, 512], F32, name="o_ps")
                            for dt in range(ET):
                                nc.tensor.matmul(
                                    o_ps,
                                    av_sb[:, dt, j * P : (j + 1) * P],
                                    vo_sb[:, dt, eh * 512 : (eh + 1) * 512],
                                    start=(dt == 0),
                                    stop=(dt == ET - 1),
                                )
                            nc.vector.tensor_scalar_mul(
                                out_sb[:, eh * 512 : (eh + 1) * 512],
                                o_ps,
                                r_sb[:, j : j + 1],
                            )
                        nc.sync.dma_start(
                            out[q0 + j * P : q0 + (j + 1) * P, :], out_sb
                        )

    nc.compile()
    return nc


_NC = None


def _get_nc():
    global _NC
    if _NC is None:
        _NC = _build()
    return _NC


_RUNNER = None


def _get_runner():
    """Cached jitted 8-core SPMD executor (avoids re-tracing per call)."""
    global _RUNNER
    if _RUNNER is not None:
        return _RUNNER

    import jax
    from jax.sharding import Mesh, NamedSharding, PartitionSpec
    from jax.experimental.shard_map import shard_map
    from concourse import bass2jax

    nc = _get_nc()
    bass2jax.install_neuronx_cc_hook()
    partition_name = nc.partition_id_tensor.name if nc.partition_id_tensor else None
    in_names, out_names, out_avals, zero_outs = [], [], [], []
    for alloc in nc.m.functions[0].allocations:
        if not isinstance(alloc, mybir.MemoryLocationSet):
            continue
        name = alloc.memorylocations[0].name
        if alloc.kind == "ExternalInput":
            if name != partition_name:
                in_names.append(name)
        elif alloc.kind == "ExternalOutput":
            shape = tuple(alloc.tensor_shape)
            dtype = mybir.dt.np(alloc.dtype)
            out_names.append(name)
            out_avals.append(jax.core.ShapedArray(shape, dtype))
            zero_outs.append(np.zeros(shape, dtype))
    n_params = len(in_names)
    n_outs = len(out_avals)
    in_names_all = in_names + out_names
    if partition_name is not None:
        in_names_all = in_names_all + [partition_name]

    def _body(*args):
        operands = list(args)
        if partition_name is not None:
            operands.append(bass2jax.partition_id_tensor())
        return tuple(
            bass2jax._bass_exec_p.bind(
                *operands,
                out_avals=tuple(out_avals),
                in_names=tuple(in_names_all),
                out_names=tuple(out_names),
                lowering_input_output_aliases=(),
                sim_require_finite=True,
                sim_require_nnan=True,
                nc=nc,
            )
        )

    devices = jax.devices()[:8]
    mesh = Mesh(np.asarray(devices), ("core",))
    in_specs = (PartitionSpec("core"),) * (n_params + n_outs)
    out_specs = (PartitionSpec("core"),) * len(out_names)
    donate = tuple(range(n_params, n_params + n_outs))
    sharded = jax.jit(
        shard_map(
            _body, mesh=mesh, in_specs=in_specs, out_specs=out_specs, check_rep=False
        ),
        donate_argnums=donate,
        keep_unused=True,
    )
    shard = NamedSharding(mesh, PartitionSpec("core"))

    def run(in_maps):
        per_core = [[np.asarray(m[nm]) for nm in in_names] for m in in_maps]
        concat_in = [
            np.concatenate([per_core[c][i] for c in range(8)], axis=0)
            for i in range(n_params)
        ]
        concat_zeros = [
            np.zeros((8 * z.shape[0], *z.shape[1:]), z.dtype) for z in zero_outs
        ]
        out_arrs = sharded(*concat_in, *concat_zeros)
        return [
            {
                name: np.asarray(out_arrs[i]).reshape(8, *out_avals[i].shape)[c]
                for i, name in enumerate(out_names)
            }
            for c in range(8)
        ]

    _RUNNER = run
    return _RUNNER


def _make_in_maps(inputs):
    x = np.asarray(inputs["x"], dtype=np.float32)
    QK16 = np.asarray(inputs["QK"], dtype=np.float32).astype(NPBF16)
    VO16 = np.asarray(inputs["VO"], dtype=np.float32).astype(NPBF16)
    in_maps = []
    for c in range(8):
        b, h = divmod(c, 2)
        xb16 = x[b].astype(NPBF16)                         # [T, D]
        xTb16 = np.ascontiguousarray(xb16.T)               # [D, T]
        in_maps.append(
            {
                "x": xb16,
                "xT": xTb16,
                "xTq": np.ascontiguousarray(xTb16[:, h * TQ : (h + 1) * TQ]),
                "QK": QK16,
                "VO": VO16,
            }
        )
    return in_maps


def kernel(x, QK, VO):
    in_maps = _make_in_maps({"x": x, "QK": QK, "VO": VO})
    results = _get_runner()(in_maps)
    out = np.empty((B, T, D), dtype=np.float32)
    for c in range(8):
        b, h = divmod(c, 2)
        out[b, h * TQ : (h + 1) * TQ, :] = results[c]["out"]
    return out
